# revision 16
# baseline (speedup 1.0000x reference)
"""Mixtral MoE MLP (ragged grouped-GEMM SwiGLU) on 8 Trainium2 NeuronCores.

Sharding: tensor-parallel over the intermediate dim DI. Core c owns the
DI columns [c*DI/8, (c+1)*DI/8) of w1/w3 (and the matching rows of w2)
for ALL experts. Every core processes ALL tokens -> the Bass program is
identical across cores (true SPMD; only the weight data differs per
core). Each core produces a partial y (rank-DI/8 contribution); the
8-way sum ("all-reduce after w2") is done on the host after gather.

Per-core compute: 3 * 2*NT*DH*(DI/8) = ~25.8 GFLOP, bf16 matmuls with
fp32 PSUM accumulation.

Layouts (host-packed so every DMA is one large fully-contiguous
transfer; strided DRAM access patterns shatter into ~1KB descriptors
and saturate the SDMA rings):
  xc{i} [128, DH/128, n_i]        xc[p,k,t]    = x[t0_i+t, k*128+p]   (bf16)
  w1h [NE, 128, DH/128, DI_SH]    w1h[e,p,k,j] = w1[e, k*128+p, dlo+j]
  w3h same as w1h
  w2h [NE, 128, DI_SH/128, DH]    w2h[e,p,k,j] = w2[e, dlo+k*128+p, j]
  yc{i} [128, DH/128, n_i]        yc[p,m,t]    = y_partial[t0_i+t, m*128+p]
where dlo = c*DI_SH is this core's DI offset and (t0_i, n_i) is chunk
i's token range.
"""

import os
import sys

import numpy as np
import ml_dtypes

for _p in ("/opt/trn_rl_repo", "/root/.axon_site/_ro/trn_rl_repo"):
    if os.path.isdir(_p) and _p not in sys.path:
        sys.path.append(_p)

import concourse.bass as bass  # noqa: E402
import concourse.bacc as bacc  # noqa: E402
import concourse.tile as tile  # noqa: E402
import concourse.mybir as mybir  # noqa: E402
from concourse.bass_utils import run_bass_kernel_spmd  # noqa: E402


def _ensure_ntff_hook_shim():
    """concourse's trace path imports antenv.axon_hooks, which this image
    lacks; provide a functional stand-in so tracing works (or degrades
    gracefully) instead of raising ImportError."""
    try:
        import antenv.axon_hooks  # noqa: F401
        return
    except Exception:
        pass
    import types

    try:
        import antenv
    except Exception:
        antenv = types.ModuleType("antenv")
        sys.modules["antenv"] = antenv
    mod = types.ModuleType("antenv.axon_hooks")
    state = {"hook": None, "tried": False}

    def set_axon_ntff_profile_hook(h):
        state["hook"] = h

    def get_axon_ntff_profile_hook():
        if state["hook"] is None and not state["tried"]:
            state["tried"] = True
            try:
                from trn_agent_boot.trn_boot import _ntff_profile_via_ctypes

                state["hook"] = _ntff_profile_via_ctypes(
                    "/opt/axon/libaxon_pjrt.so"
                )
            except Exception:
                state["hook"] = None
        return state["hook"]

    mod.set_axon_ntff_profile_hook = set_axon_ntff_profile_hook
    mod.get_axon_ntff_profile_hook = get_axon_ntff_profile_hook
    sys.modules["antenv.axon_hooks"] = mod
    antenv.axon_hooks = mod


_ensure_ntff_hook_shim()

BF16 = mybir.dt.bfloat16
F32 = mybir.dt.float32
NPBF16 = ml_dtypes.bfloat16

N_CORES = 8
P = 128
NMAX = 512  # max matmul moving free dim (one PSUM bank of fp32)

# Knobs for experimentation from test.py
TRACE = False
TRACE_CORES = None
LAST_RESULTS = None

_prog_cache: dict = {}


def _plan_chunks(group_sizes):
    """Split each expert's token range into near-equal chunks of <= NMAX.

    Returns (chunks, nt_eff) where chunks is a list of (expert, t0, n).
    Chunks are emitted grouped by expert, in a schedule order chosen so
    that (a) the first expert is small (short time-to-first-matmul),
    (b) each expert's weight load is covered by a preceding large
    compute window, (c) the last chunk is small (short output drain).
    """
    per_e = {}
    off = 0
    for e, g in enumerate(group_sizes):
        g = int(g)
        if g > 0:
            k = -(-g // NMAX)
            base, rem = divmod(g, k)
            t = off
            cl = []
            for i in range(k):
                n = base + (1 if i < rem else 0)
                cl.append((e, t, n))
                t += n
            per_e[e] = cl
        off += g

    # Biggest experts first: their long compute windows cover successor
    # weight loads; the expert with the smallest final chunk goes last
    # (short output drain before the exit barrier).
    order = sorted(per_e, key=lambda e: -sum(c[2] for c in per_e[e]))
    if len(order) > 1:
        last = min(order, key=lambda e: per_e[e][-1][2])
        order.remove(last)
        order.append(last)

    # Carve a small leading chunk off the first expert: real matmuls can
    # then start as soon as ~0.6MB lands, and the HAM clock ramp happens
    # on useful work instead of on warm-up filler.
    FIRST = 128
    e0 = order[0]
    g0 = sum(c[2] for c in per_e[e0])
    if g0 > FIRST + NMAX // 2:
        t0 = per_e[e0][0][1]
        rest = g0 - FIRST
        k = -(-rest // NMAX)
        base, rem = divmod(rest, k)
        cl = [(e0, t0, FIRST)]
        t = t0 + FIRST
        for i in range(k):
            n = base + (1 if i < rem else 0)
            cl.append((e0, t, n))
            t += n
        per_e[e0] = cl

    chunks = [c for e in order for c in per_e[e]]
    return chunks, off


W2G = 512  # w2 blocks carry this many DH columns per DMA


def _build_program(ne, dh, di_sh, nt, chunks):
    kd = dh // P     # k-tiles for gemm1/3 (contraction over DH)
    md = di_sh // P  # m-tiles for gemm1/3 == k-tiles for gemm2
    mo = dh // P     # m-tiles for gemm2 (output DH)
    gh = dh // W2G   # w2 DMA groups (W2G//P m-tiles per group)
    gm = W2G // P

    nc = bacc.Bacc(
        "TRN2", target_bir_lowering=False, debug=False, num_devices=N_CORES
    )
    xc = [
        nc.dram_tensor(f"xc{ci}", [P, kd, n], BF16, kind="ExternalInput")
        for ci, (e, t0, n) in enumerate(chunks)
    ]
    w1h = nc.dram_tensor("w1h", [ne, md, P, kd, P], BF16, kind="ExternalInput")
    w3h = nc.dram_tensor("w3h", [ne, md, P, kd, P], BF16, kind="ExternalInput")
    w2h = nc.dram_tensor("w2h", [ne, gh, P, md, W2G], BF16, kind="ExternalInput")
    yc = [
        nc.dram_tensor(f"yc{ci}", [P, mo, n], BF16, kind="ExternalOutput")
        for ci, (e, t0, n) in enumerate(chunks)
    ]

    silu = mybir.ActivationFunctionType.Silu

    with tile.TileContext(nc) as tc:
        with (
            tc.tile_pool(name="w1pool", bufs=8) as w1pool,
            tc.tile_pool(name="w3pool", bufs=8) as w3pool,
            tc.tile_pool(name="w2pool", bufs=8) as w2pool,
            tc.tile_pool(name="xpool", bufs=2) as xpool,
            tc.tile_pool(name="hpool", bufs=2) as hpool,
            tc.tile_pool(name="cpool", bufs=4) as cpool,
            tc.tile_pool(name="opool", bufs=2) as opool,
            tc.tile_pool(name="psh", bufs=2, space="PSUM") as psh,
            tc.tile_pool(name="psy", bufs=4, space="PSUM") as psy,
        ):
            # PE pre-warm: dummy matmuls on a zeroed scratch tile run
            # during the initial DMA wait, so the HAM clock-gate reaches
            # K=8/8 (2.4 GHz) before the first real matmul issues.
            warm_sb = cpool.tile([P, P], BF16, tag="warm")
            nc.gpsimd.memset(warm_sb[:], 0.0)
            # Bridge the ~1.5us between PE boot and chunk 0's first data
            # with dummy MMs; the (small) first chunk then ramps HAM on
            # real work.
            warm_ps = psy.tile([P, P], F32, tag="y", name="warm_ps")
            for _ in range(16):
                nc.tensor.matmul(warm_ps[:], warm_sb[:], warm_sb[:])

            # DMA emission runs one chunk ahead of compute so the next
            # chunk's weights/x are enqueued on the SP ring BEFORE the
            # current chunk's output DMAs (FIFO ring: otherwise the next
            # expert's first loads start only after the last out drains).
            wtiles = {}

            def emit_loads(ci):
                e, t0, n = chunks[ci]
                first = ci == 0 or chunks[ci - 1][0] != e
                if first:
                    w1b = [
                        w1pool.tile([P, kd, P], BF16, tag="w1", name=f"w1b{e}_{i}")
                        for i in range(md)
                    ]
                    w3b = [
                        w3pool.tile([P, kd, P], BF16, tag="w3", name=f"w3b{e}_{i}")
                        for i in range(md)
                    ]
                    w2b = [
                        w2pool.tile([P, md, W2G], BF16, tag="w2", name=f"w2b{e}_{i}")
                        for i in range(gh)
                    ]
                    wtiles[e] = (w1b, w3b, w2b)
                x_sb = xpool.tile([P, kd, n], BF16, tag="x", name=f"x{ci}")
                w1b, w3b, w2b = wtiles[e]
                if ci == 0:
                    # critical path: first GEMM needs only w1 block 0 k-tile 0
                    # + x k-tile 0; quarter-split both so the first matmul
                    # starts after ~0.6 MB instead of ~1.3 MB.
                    kq = kd // 4
                    for q in range(4):
                        ks = slice(q * kq, (q + 1) * kq)
                        nc.sync.dma_start(w1b[0][:, ks, :], w1h[e, 0, :, ks])
                        nc.sync.dma_start(x_sb[:, ks, :], xc[0][:, ks, :])
                    nc.sync.dma_start(w3b[0][:], w3h[e, 0])
                    # interleaved by m to match GEMM1/3 consumption order
                    for mi in range(1, md):
                        nc.sync.dma_start(w1b[mi][:], w1h[e, mi])
                        nc.sync.dma_start(w3b[mi][:], w3h[e, mi])
                    for g in range(gh):
                        nc.sync.dma_start(w2b[g][:], w2h[e, g])
                else:
                    if first:
                        # first m-block of w1/w3 ahead of x (the expert's
                        # first GEMM needs only these), the rest after x.
                        nc.sync.dma_start(w1b[0][:], w1h[e, 0])
                        nc.sync.dma_start(w3b[0][:], w3h[e, 0])
                    nc.sync.dma_start(x_sb[:], xc[ci][:])
                    if first:
                        for mi in range(1, md):
                            nc.sync.dma_start(w1b[mi][:], w1h[e, mi])
                            nc.sync.dma_start(w3b[mi][:], w3h[e, mi])
                        for g in range(gh):
                            nc.sync.dma_start(w2b[g][:], w2h[e, g])
                return x_sb

            x_pending = {0: emit_loads(0)}
            for ci, (e, t0, n) in enumerate(chunks):
                if ci + 1 < len(chunks):
                    x_pending[ci + 1] = emit_loads(ci + 1)
                x_sb = x_pending.pop(ci)
                w1b, w3b, w2b = wtiles[e]

                h_sb = hpool.tile([P, md, n], BF16, tag="h")
                for mi in range(md):
                    ps1 = psh.tile([P, n], F32, tag="h1")
                    for k in range(kd):
                        nc.tensor.matmul(
                            ps1[:],
                            w1b[mi][:, k, :],
                            x_sb[:, k, :],
                            start=(k == 0),
                            stop=(k == kd - 1),
                        )
                    ps3 = psh.tile([P, n], F32, tag="h3")
                    for k in range(kd):
                        nc.tensor.matmul(
                            ps3[:],
                            w3b[mi][:, k, :],
                            x_sb[:, k, :],
                            start=(k == 0),
                            stop=(k == kd - 1),
                        )
                    # silu(h1) * h3 via the HW Silu LUT (one ACT + one mul;
                    # accuracy verified identical to sigmoid+muls on HW)
                    sl = cpool.tile([P, n], F32, tag="silu")
                    nc.scalar.activation(sl[:], ps1[:], silu)
                    nc.vector.tensor_mul(h_sb[:, mi, :], sl[:], ps3[:])

                # GEMM2 with split-k emission: the first 3 m-tiles run
                # k=0..md-2 before anything touches h k-tile md-1 (whose
                # SwiGLU finishes last), hiding the ACT/DVE latency tail.
                NPRE = 3
                pre = []
                for m in range(min(NPRE, mo)):
                    psy_t = psy.tile([P, n], F32, tag="y", name=f"ypre{m}")
                    for k in range(md - 1):
                        nc.tensor.matmul(
                            psy_t[:],
                            w2b[m // gm][:, k, (m % gm) * P : (m % gm + 1) * P],
                            h_sb[:, k, :],
                            start=(k == 0),
                            stop=False,
                        )
                    pre.append(psy_t)
                # Output is staged in SBUF and written back as two ~1MB
                # contiguous DMAs per chunk (strided [P,n] stores into a
                # [mo,P,nt] tensor cost ~77ns/KB in descriptor churn).
                mh = mo // 2
                o_lo = opool.tile([P, mh, n], BF16, tag="olo")
                o_hi = opool.tile([P, mh, n], BF16, tag="ohi")
                for m in range(mo):
                    if m < len(pre):
                        psy_t = pre[m]
                        nc.tensor.matmul(
                            psy_t[:],
                            w2b[m // gm][:, md - 1, (m % gm) * P : (m % gm + 1) * P],
                            h_sb[:, md - 1, :],
                            start=False,
                            stop=True,
                        )
                    else:
                        psy_t = psy.tile([P, n], F32, tag="y")
                        for k in range(md):
                            nc.tensor.matmul(
                                psy_t[:],
                                w2b[m // gm][:, k, (m % gm) * P : (m % gm + 1) * P],
                                h_sb[:, k, :],
                                start=(k == 0),
                                stop=(k == md - 1),
                            )
                    osb = o_lo if m < mh else o_hi
                    nc.vector.tensor_copy(osb[:, m % mh, :], psy_t[:])
                    # drain output in quarters so the post-last-matmul DMA
                    # tail is ~0.25MB, not 1MB
                    if (m + 1) % (mh // 2) == 0 and m + 1 < mo:
                        q = (m + 1) // (mh // 2) - 1
                        qs = slice(q * (mh // 2) % mh, q * (mh // 2) % mh + mh // 2)
                        nc.sync.dma_start(yc[ci][:, (m + 1 - mh // 2) : (m + 1), :], osb[:, qs, :])
                nc.sync.dma_start(yc[ci][:, mo - mh // 2 :, :], o_hi[:, mh // 2 :, :])

    nc.compile()
    return nc


def _get_program(ne, dh, di, nt_eff, chunk_key):
    key = (ne, dh, di, nt_eff, chunk_key)
    prog = _prog_cache.get(key)
    if prog is None:
        chunks = [tuple(c) for c in chunk_key]
        prog = _build_program(ne, dh, di // N_CORES, nt_eff, chunks)
        _prog_cache[key] = prog
    return prog


def kernel(x, group_sizes, w1, w2, w3):
    x = np.asarray(x, dtype=np.float32)
    group_sizes = np.asarray(group_sizes)
    w1 = np.asarray(w1, dtype=np.float32)
    w2 = np.asarray(w2, dtype=np.float32)
    w3 = np.asarray(w3, dtype=np.float32)

    ne, dh, di = w1.shape
    di_sh = di // N_CORES
    kd = dh // P
    md = di_sh // P

    chunks, nt_eff = _plan_chunks(group_sizes)
    if nt_eff == 0:
        return np.zeros((0, dh), dtype=np.float32)

    nc = _get_program(ne, dh, di, nt_eff, tuple(chunks))

    # ---- host-side pack / shard ----
    # per-chunk xc[p, k, t] = x[t0+t, k*128+p], contiguous per chunk
    xbf = x[:nt_eff].astype(NPBF16)
    xc_arrs = {
        f"xc{ci}": np.ascontiguousarray(
            xbf[t0 : t0 + n].reshape(n, kd, P).transpose(2, 1, 0)
        )
        for ci, (e, t0, n) in enumerate(chunks)
    }
    # w1/w3: [NC, NE, MD, P(part), KD, P(col)]
    # dh = k*128+p ; di = c*di_sh + mi*128 + j
    w1a = np.ascontiguousarray(
        w1.astype(NPBF16)
        .reshape(ne, kd, P, N_CORES, md, P)
        .transpose(3, 0, 4, 2, 1, 5)
    )
    w3a = np.ascontiguousarray(
        w3.astype(NPBF16)
        .reshape(ne, kd, P, N_CORES, md, P)
        .transpose(3, 0, 4, 2, 1, 5)
    )
    # w2: [NC, NE, GH, P(part), MD, W2G]
    # di = c*di_sh + k*128 + p ; dh = g*W2G + j
    gh = dh // W2G
    w2a = np.ascontiguousarray(
        w2.astype(NPBF16)
        .reshape(ne, N_CORES, md, P, gh, W2G)
        .transpose(1, 0, 4, 3, 2, 5)
    )

    in_maps = [
        {**xc_arrs, "w1h": w1a[c], "w3h": w3a[c], "w2h": w2a[c]}
        for c in range(N_CORES)
    ]

    global LAST_RESULTS
    res = run_bass_kernel_spmd(
        nc,
        in_maps,
        core_ids=list(range(N_CORES)),
        trace=TRACE,
        trace_cores=TRACE_CORES,
    )
    LAST_RESULTS = res

    # ---- host-side gather + 8-way partial sum ("all-reduce") ----
    mo = dh // P
    acc = np.zeros((dh, nt_eff), dtype=np.float32)
    for c in range(N_CORES):
        r = res.results[c]
        for ci, (e, t0, n) in enumerate(chunks):
            ycv = r[f"yc{ci}"]  # [P, mo, n] bf16; y[t0+t, m*128+p]
            acc[:, t0 : t0 + n] += (
                ycv.transpose(1, 0, 2).reshape(dh, n).astype(np.float32)
            )
    return np.ascontiguousarray(acc.T)

